# revision 1
# baseline (speedup 1.0000x reference)
"""MultiHeadCrossAttention on 8 TRN2 NeuronCores.

Sharding: tensor-parallel over heads (16 heads -> 2 per core).
All activations live transposed ([features, tokens]) on device so every
matmul contracts over the partition dim with zero on-device transposes of
the big activations (V is PE-transposed per 128-col block, which is cheap).
Per core:
  Q.T = (Wq.T slice).T @ x1.T   [128, 4096]
  K.T, V.T from x2.T            [128, 8192]
  per (batch, qcol-chunk, head): S.T = K @ Q.T ; P.T = exp(S.T/8) ;
    outT[d|den] = [V|1]-chunks.T @ P.T  (ones column gives the softmax
    denominator for free) ; attnT = outT[0:64] * recip(outT[64])
  Y.T partial = (Wo.T row-slice).T @ attnT  [1024, 4096]
Host: pre-tiles inputs for contiguous DMA, sums the 8 partials, adds bo,
transposes back. Emission is software-pipelined: KV-projection of batch
b+1 is emitted before attention of batch b; out-projection is fused per
q-column chunk right after its normalize.
"""
import numpy as np
from contextlib import ExitStack

import concourse.bass as bass
import concourse.mybir as mybir
import concourse.tile as tile
from concourse import bacc
from concourse.bass_utils import run_bass_kernel_spmd

N_CORES = 8
B, SQ, SKV, E, DH = 4, 1024, 2048, 1024, 64
Q_ROWS = B * SQ      # 4096
KV_ROWS = B * SKV    # 8192
EC = E // 128        # 8 contraction chunks
QC = Q_ROWS // 512   # 8 q column chunks
KVC_B = SKV // 128   # 16 kv chunks per batch
GB = SQ // 512       # 2 q chunks per batch
F32R = mybir.dt.float32r
F32 = mybir.dt.float32
Exp = mybir.ActivationFunctionType.Exp

_CACHE = {}


def _build(phases=("proj", "attn", "oproj"), n_reps=1):
    nc = bacc.Bacc("TRN2", target_bir_lowering=False, debug=False,
                   num_devices=N_CORES)
    # host-pretiled inputs: each [.., 128, EC, 512] slab is one contiguous DMA
    x1t = nc.dram_tensor("x1t", [QC, 128, EC, 512], F32R,
                         kind="ExternalInput").ap()
    x2t = nc.dram_tensor("x2t", [KV_ROWS // 512, 128, EC, 512], F32R,
                         kind="ExternalInput").ap()
    wqt = nc.dram_tensor("wqt", [128, EC, 128], F32R, kind="ExternalInput").ap()
    wkt = nc.dram_tensor("wkt", [128, EC, 128], F32R, kind="ExternalInput").ap()
    wvt = nc.dram_tensor("wvt", [128, EC, 128], F32R, kind="ExternalInput").ap()
    wot = nc.dram_tensor("wot", [128, E], F32R, kind="ExternalInput").ap()
    bqv = nc.dram_tensor("bq", [128, 1], F32, kind="ExternalInput").ap()
    bkv = nc.dram_tensor("bk", [128, 1], F32, kind="ExternalInput").ap()
    bvv = nc.dram_tensor("bv", [128, 1], F32, kind="ExternalInput").ap()
    idv = nc.dram_tensor("ident", [128, 128], F32R, kind="ExternalInput").ap()
    onv = nc.dram_tensor("ones", [128, 1], F32R, kind="ExternalInput").ap()
    yt = nc.dram_tensor("yt", [E, Q_ROWS], F32, kind="ExternalOutput").ap()
    yt_r = yt.rearrange("(oc p) q -> p oc q", p=128)

    do_proj = "proj" in phases
    do_attn = "attn" in phases and do_proj
    do_oproj = "oproj" in phases and do_attn

    with tile.TileContext(nc) as tc, ExitStack() as ctx:
        const = ctx.enter_context(tc.tile_pool(name="const", bufs=1))
        persist = ctx.enter_context(tc.tile_pool(name="persist", bufs=1))
        xload = ctx.enter_context(tc.tile_pool(name="xload", bufs=7))
        work = ctx.enter_context(tc.tile_pool(name="work", bufs=3))
        ps_pj = ctx.enter_context(tc.tile_pool(name="ps_pj", bufs=2, space="PSUM"))
        ps_s = ctx.enter_context(tc.tile_pool(name="ps_s", bufs=2, space="PSUM"))
        ps_o = ctx.enter_context(tc.tile_pool(name="ps_o", bufs=2, space="PSUM"))

        wq_sb = const.tile([128, EC, 128], F32R, tag="wq")
        wk_sb = const.tile([128, EC, 128], F32R, tag="wk")
        wv_sb = const.tile([128, EC, 128], F32R, tag="wv")
        wo_sb = const.tile([128, E], F32R, tag="wo")
        bq_sb = const.tile([128, 1], F32, tag="bq")
        bk_sb = const.tile([128, 1], F32, tag="bk")
        bv_sb = const.tile([128, 1], F32, tag="bv")
        id_sb = const.tile([128, 128], F32R, tag="id")
        ones_sb = const.tile([128, 1], F32R, tag="ones1")
        nc.sync.dma_start(wq_sb[:], wqt[:])
        nc.sync.dma_start(wk_sb[:], wkt[:])
        nc.sync.dma_start(wv_sb[:], wvt[:])
        nc.sync.dma_start(wo_sb[:], wot[:])
        nc.sync.dma_start(bq_sb[:], bqv[:])
        nc.sync.dma_start(bk_sb[:], bkv[:])
        nc.sync.dma_start(bv_sb[:], bvv[:])
        nc.sync.dma_start(id_sb[:], idv[:])
        nc.sync.dma_start(ones_sb[:], onv[:])

        for rep in range(n_reps):
            qt_sb = persist.tile([128, Q_ROWS], F32R, tag="qt", name=f"qt_{rep}")
            kt_sb = [persist.tile([128, SKV], F32R, tag=f"kt{b}",
                                  name=f"kt{b}_{rep}") for b in range(B)]
            v_sb = [persist.tile([128, KVC_B, 130], F32R, tag=f"v{b}",
                                 name=f"v{b}_{rep}") for b in range(B)]
            at_sb = [persist.tile([128, SQ], F32R, tag=f"at{b}",
                                  name=f"atz{b}_{rep}") for b in range(B)]

            def proj_q(j):
                for u in range(2):
                    xt = xload.tile([128, EC, 256], F32R, tag="x",
                                    name=f"xq{j}_{u}_{rep}")
                    nc.sync.dma_start(xt[:], x1t[j][:, :, u * 256:(u + 1) * 256])
                    if not do_proj:
                        continue
                    q_ps = ps_pj.tile([128, 256], F32, tag="pj",
                                      name=f"qps{j}_{u}_{rep}")
                    for ec in range(EC):
                        nc.tensor.matmul(q_ps[:], wq_sb[:, ec], xt[:, ec],
                                         start=(ec == 0), stop=(ec == EC - 1))
                    c0 = j * 512 + u * 256
                    nc.vector.tensor_scalar_add(qt_sb[:, c0:c0 + 256],
                                                q_ps[:], bq_sb[:])

            def proj_kv(b, half=None):
                rng = range(SKV // 512) if half is None else \
                    range(half * (SKV // 1024), (half + 1) * (SKV // 1024))
                for jj in rng:
                    j = b * (SKV // 512) + jj
                    for u in range(2):
                        xt = xload.tile([128, EC, 256], F32R, tag="x",
                                        name=f"xt{b}_{jj}_{u}_{rep}")
                        nc.sync.dma_start(xt[:],
                                          x2t[j][:, :, u * 256:(u + 1) * 256])
                        if not do_proj:
                            continue
                        k_ps = ps_pj.tile([128, 256], F32, tag="pj",
                                          name=f"kps{b}_{jj}_{u}_{rep}")
                        for ec in range(EC):
                            nc.tensor.matmul(k_ps[:], wk_sb[:, ec], xt[:, ec],
                                             start=(ec == 0), stop=(ec == EC - 1))
                        c0 = jj * 512 + u * 256
                        nc.vector.tensor_scalar_add(
                            kt_sb[b][:, c0:c0 + 256], k_ps[:], bk_sb[:])
                        v_ps = ps_pj.tile([128, 256], F32, tag="pj",
                                          name=f"vps{b}_{jj}_{u}_{rep}")
                        for ec in range(EC):
                            nc.tensor.matmul(v_ps[:], wv_sb[:, ec], xt[:, ec],
                                             start=(ec == 0), stop=(ec == EC - 1))
                        vt_tmp = work.tile([128, 256], F32R, tag="vt", bufs=3,
                                           name=f"vtt{b}_{jj}_{u}_{rep}")
                        nc.vector.tensor_scalar_add(vt_tmp[:], v_ps[:], bv_sb[:])
                        for t in range(2):
                            kc = jj * 4 + u * 2 + t
                            vtp = ps_pj.tile([128, 128], F32R, tag="pj",
                                             name=f"vtp{b}_{kc}_{rep}")
                            nc.tensor.transpose(vtp[:],
                                                vt_tmp[:, t * 128:(t + 1) * 128],
                                                id_sb[:])
                            dst = v_sb[b][:, kc].rearrange("p (h x) -> p h x",
                                                           h=2)
                            nc.vector.tensor_copy(
                                dst[:, :, 0:64],
                                vtp[:].rearrange("p (h x) -> p h x", h=2))

            def oproj_g(b, g):
                if not do_oproj:
                    return
                for o in range(EC):
                    y_ps = ps_pj.tile([128, 512], F32, tag="pj",
                                      name=f"yps{b}_{g}_{o}_{rep}")
                    nc.tensor.matmul(y_ps[:], wo_sb[:, o * 128:(o + 1) * 128],
                                     at_sb[b][:, g * 512:(g + 1) * 512],
                                     start=True, stop=True)
                    y_sb = work.tile([128, 512], F32, tag="y", bufs=3,
                                     name=f"ysb{b}_{g}_{o}_{rep}")
                    nc.vector.tensor_copy(y_sb[:], y_ps[:])
                    nc.sync.dma_start(
                        yt_r[:, o, b * SQ + g * 512: b * SQ + (g + 1) * 512],
                        y_sb[:])

            def attn(b, gsel=None):
                if not do_attn:
                    return
                if gsel in (None, 0):
                    vv = v_sb[b][:].rearrange("p kc (h x) -> p (kc h) x", x=65)
                    nc.vector.tensor_copy(vv[:, :, 64:65],
                                          ones_sb[:].unsqueeze(-1)
                                          .to_broadcast((128, 2 * KVC_B, 1)))
                for g in range(GB) if gsel is None else [gsel]:
                    gs = slice(g * 512, (g + 1) * 512)
                    o_ps = [ps_o.tile([65, 512], F32, tag="o",
                                      name=f"o{b}_{g}_{h}_{rep}")
                            for h in range(2)]
                    for kc in range(0, KVC_B, 2):
                        for h in range(2):
                            hp = h * 64
                            s_ps = ps_s.tile([128, 1024], F32, tag="s",
                                             name=f"sps{b}_{g}_{kc}_{h}_{rep}")
                            pt = work.tile([128, 1024], F32R, tag="pt", bufs=4,
                                           name=f"pt{b}_{g}_{kc}_{h}_{rep}")
                            for u in range(2):
                                nc.tensor.matmul(
                                    s_ps[:, u * 512:(u + 1) * 512],
                                    kt_sb[b][hp:hp + 64,
                                             (kc + u) * 128:(kc + u + 1) * 128],
                                    qt_sb[hp:hp + 64, b * SQ + g * 512:
                                          b * SQ + (g + 1) * 512],
                                    start=True, stop=True)
                            nc.scalar.activation(pt[:], s_ps[:], Exp,
                                                 scale=0.125)
                            for u in range(2):
                                nc.tensor.matmul(
                                    o_ps[h][:],
                                    v_sb[b][:, kc + u, h * 65:h * 65 + 65],
                                    pt[:, u * 512:(u + 1) * 512],
                                    start=(kc == 0 and u == 0),
                                    stop=(kc == KVC_B - 2 and u == 1))
                    for h in range(2):
                        hp = h * 64
                        recip = work.tile([1, 512], F32, tag="recip", bufs=2,
                                          name=f"rc{b}_{g}_{h}_{rep}")
                        nc.vector.reciprocal(recip[:], o_ps[h][64:65, :])
                        rbc = work.tile([64, 512], F32, tag="rbc", bufs=2,
                                        name=f"rbc{b}_{g}_{h}_{rep}")
                        nc.gpsimd.partition_broadcast(rbc[:], recip[:])
                        nc.vector.tensor_mul(at_sb[b][hp:hp + 64, gs],
                                             o_ps[h][0:64, :], rbc[:])
                    oproj_g(b, g)

            # software-pipelined emission: proj(b+1) ahead of attn(b),
            # Q chunks just-in-time (attn(b) needs chunks 2b, 2b+1)
            proj_q(0)
            proj_q(1)
            proj_kv(0)
            for b in range(B):
                if b + 1 < B:
                    proj_q(2 * b + 2)
                    proj_kv(b + 1, half=0)
                    attn(b, gsel=0)
                    proj_q(2 * b + 3)
                    proj_kv(b + 1, half=1)
                    attn(b, gsel=1)
                else:
                    attn(b)

    nc.compile()
    return nc


def _get_nc(phases=("proj", "attn", "oproj"), n_reps=1):
    key = (tuple(phases), n_reps)
    if key not in _CACHE:
        _CACHE[key] = _build(phases, n_reps)
    return _CACHE[key]


def _tile_x(xt2d, nchunks):
    # [E, R] -> [R/512, 128, EC, 512]: x[j, p, ec, q] = xt2d[ec*128+p, j*512+q]
    return np.ascontiguousarray(
        xt2d.reshape(EC, 128, nchunks, 512).transpose(2, 1, 0, 3))


def _tile_w(wt_slice):
    # [E, 128] -> [128, EC, 128]
    return np.ascontiguousarray(
        wt_slice.reshape(EC, 128, 128).transpose(1, 0, 2))


def make_in_maps(x1, x2, Wq, bq, Wk, bk, Wv, bv, Wo, bo=None):
    x1 = np.asarray(x1, dtype=np.float32)
    x2 = np.asarray(x2, dtype=np.float32)
    x1t = _tile_x(np.ascontiguousarray(x1.reshape(Q_ROWS, E).T), QC)
    x2t = _tile_x(np.ascontiguousarray(x2.reshape(KV_ROWS, E).T),
                  KV_ROWS // 512)
    WqT = np.asarray(Wq, dtype=np.float32).T
    WkT = np.asarray(Wk, dtype=np.float32).T
    WvT = np.asarray(Wv, dtype=np.float32).T
    WoT = np.ascontiguousarray(np.asarray(Wo, dtype=np.float32).T)
    ident = np.eye(128, dtype=np.float32)
    ones = np.ones((128, 1), dtype=np.float32)
    in_maps = []
    for c in range(N_CORES):
        s = slice(128 * c, 128 * (c + 1))
        in_maps.append({
            "x1t": x1t, "x2t": x2t,
            "wqt": _tile_w(WqT[:, s]),
            "wkt": _tile_w(WkT[:, s]),
            "wvt": _tile_w(WvT[:, s]),
            "wot": np.ascontiguousarray(WoT[s, :]),
            "bq": np.ascontiguousarray(
                np.asarray(bq, np.float32)[s]).reshape(128, 1),
            "bk": np.ascontiguousarray(
                np.asarray(bk, np.float32)[s]).reshape(128, 1),
            "bv": np.ascontiguousarray(
                np.asarray(bv, np.float32)[s]).reshape(128, 1),
            "ident": ident, "ones": ones,
        })
    return in_maps


def kernel(x1, x2, Wq, bq, Wk, bk, Wv, bv, Wo, bo):
    nc = _get_nc()
    in_maps = make_in_maps(x1, x2, Wq, bq, Wk, bk, Wv, bv, Wo)
    res = run_bass_kernel_spmd(nc, in_maps, list(range(N_CORES)))
    ytf = res.results[0]["yt"].astype(np.float64)
    for c in range(1, N_CORES):
        ytf += res.results[c]["yt"]
    y = ytf.T.astype(np.float32) + np.asarray(bo, np.float32)[None, :]
    return y.reshape(B, SQ, E)



# revision 4
# speedup vs baseline: 1.2605x; 1.2605x over previous
"""MultiHeadCrossAttention on 8 TRN2 NeuronCores.

Sharding: tensor-parallel over heads (16 heads -> 2 per core).
All matmul inputs are fp16 (host-converted), halving HBM traffic; PSUM
accumulation stays fp32. Bias algebra: bk vanishes under softmax (it only
adds a per-query constant), bv folds into the output bias on host
(softmax rows sum to 1), so only bq is applied on device.
Per core:
  Q.T = (Wq.T slice).T @ x1.T + bq     [128, 4096]  (features x tokens)
  K.T from x2.T                        [128, 8192]
  V    projected directly in [kv, d] natural layout (lhsT = x2.T chunk),
       stored per 128-kv chunk as [kv, 2*(64+1)] with a ones column that
       yields the softmax denominator for free.
  scores phase (b,g,h,kc): S.T = K @ Q.T in PSUM; P.T = exp(S.T/8) fp16,
       buffered in SBUF for the whole (b,g) block.
  consumer phase: O[q, d|den] = sum_kv P.T-chunk.T @ V-chunk, one
       CONTIGUOUS accumulation group per PSUM bank (matmul start= clears
       the whole bank's has_written bits, so groups must not interleave
       within a bank); normalize A = O[:, :64]*recip(O[:, 64]);
       PE-transpose -> A.T ; Y.T partial = (Wo.T row-slice).T @ A.T.
Macro-pipeline: phase (b,g)'s consumers are emitted interleaved into
phase (b,g)+1's scores/exp stream so the Act engine (exp) and PE stay
concurrently busy; projections of batch b+1 are chopped into quanta and
interleaved the same way. Host: pre-tiles/converts inputs, sums the 8
fp16 partials in fp32, adds bo_eff = bo + Wo @ bv, transposes back.
"""
import numpy as np
from contextlib import ExitStack

import concourse.bass as bass
import concourse.mybir as mybir
import concourse.tile as tile
from concourse import bacc
from concourse.bass_utils import run_bass_kernel_spmd

N_CORES = 8
B, SQ, SKV, E, DH = 4, 1024, 2048, 1024, 64
Q_ROWS = B * SQ      # 4096
KV_ROWS = B * SKV    # 8192
EC = E // 128        # 8 contraction chunks
QC = Q_ROWS // 512   # 8 q column chunks
KVC_B = SKV // 128   # 16 kv chunks per batch
F16 = mybir.dt.float16
F32 = mybir.dt.float32
Exp = mybir.ActivationFunctionType.Exp

_CACHE = {}


def _build(phases=("proj", "attn", "oproj"), n_reps=1):
    nc = bacc.Bacc("TRN2", target_bir_lowering=False, debug=False,
                   num_devices=N_CORES)
    x1t = nc.dram_tensor("x1t", [QC, 128, EC, 512], F16,
                         kind="ExternalInput").ap()
    x2t = nc.dram_tensor("x2t", [KV_ROWS // 512, 128, EC, 512], F16,
                         kind="ExternalInput").ap()
    wqt = nc.dram_tensor("wqt", [128, EC, 128], F16, kind="ExternalInput").ap()
    wkt = nc.dram_tensor("wkt", [128, EC, 128], F16, kind="ExternalInput").ap()
    wvt = nc.dram_tensor("wvt", [128, EC, 128], F16, kind="ExternalInput").ap()
    wot = nc.dram_tensor("wot", [128, E], F16, kind="ExternalInput").ap()
    bqv = nc.dram_tensor("bq", [128, 1], F32, kind="ExternalInput").ap()
    idv = nc.dram_tensor("ident", [128, 128], F16, kind="ExternalInput").ap()
    yt = nc.dram_tensor("yt", [E, Q_ROWS], F16, kind="ExternalOutput").ap()
    yt_r = yt.rearrange("(oc p) q -> p oc q", p=128)

    do_proj = "proj" in phases
    do_attn = "attn" in phases and do_proj
    do_oproj = "oproj" in phases and do_attn

    with tile.TileContext(nc) as tc, ExitStack() as ctx:
        const = ctx.enter_context(tc.tile_pool(name="const", bufs=1))
        persist = ctx.enter_context(tc.tile_pool(name="persist", bufs=1))
        xload = ctx.enter_context(tc.tile_pool(name="xload", bufs=7))
        work = ctx.enter_context(tc.tile_pool(name="work", bufs=3))
        ps_pj = ctx.enter_context(tc.tile_pool(name="ps_pj", bufs=2, space="PSUM"))
        ps_s = ctx.enter_context(tc.tile_pool(name="ps_s", bufs=2, space="PSUM"))
        ps_o = ctx.enter_context(tc.tile_pool(name="ps_o", bufs=2, space="PSUM"))

        wq_sb = const.tile([128, EC, 128], F16, tag="wq")
        wk_sb = const.tile([128, EC, 128], F16, tag="wk")
        wv_sb = const.tile([128, EC, 128], F16, tag="wv")
        wo_sb = const.tile([128, E], F16, tag="wo")
        bq_sb = const.tile([128, 1], F32, tag="bq")
        id_sb = const.tile([128, 128], F16, tag="id")
        # DMA priority order: first Q-proj needs only wq+bq; wk/wv before
        # the first kv quantum; wo/id not until the first consumer phase.
        nc.sync.dma_start(wq_sb[:], wqt[:])
        nc.sync.dma_start(bq_sb[:], bqv[:])

        for rep in range(n_reps):
            qt_sb = persist.tile([128, Q_ROWS], F16, tag="qt", name=f"qt_{rep}")
            kt_sb = [persist.tile([128, SKV], F16, tag=f"kt{b}",
                                  name=f"kt{b}_{rep}") for b in range(B)]
            # v_sb[b][kv, kc, h*65 + d]; column h*65+64 is ones (denominator)
            v_sb = [persist.tile([128, KVC_B, 130], F16, tag=f"v{b}",
                                 name=f"v{b}_{rep}") for b in range(B)]
            at_sb = [persist.tile([128, SQ], F16, tag=f"at{b}",
                                  name=f"atz{b}_{rep}") for b in range(B)]

            for b in range(B):
                for h in range(2):
                    c = h * 65 + 64
                    nc.vector.memset(v_sb[b][:, :, c:c + 1], 1.0)

            def proj_q(j, u):
                xt = xload.tile([128, EC, 256], F16, tag="x",
                                name=f"xq{j}_{u}_{rep}")
                nc.sync.dma_start(xt[:], x1t[j][:, :, u * 256:(u + 1) * 256])
                if not do_proj:
                    return
                q_ps = ps_pj.tile([128, 256], F32, tag="pj",
                                  name=f"qps{j}_{u}_{rep}")
                for ec in range(EC):
                    nc.tensor.matmul(q_ps[:], wq_sb[:, ec], xt[:, ec],
                                     start=(ec == 0), stop=(ec == EC - 1))
                c0 = j * 512 + u * 256
                nc.vector.tensor_scalar_add(qt_sb[:, c0:c0 + 256],
                                            q_ps[:], bq_sb[:])

            def proj_kv(b, jj, u):
                # one quantum: 256 kv tokens of batch b -> K.T rows + V chunks
                j = b * (SKV // 512) + jj
                xt = xload.tile([128, EC, 256], F16, tag="x",
                                name=f"xt{b}_{jj}_{u}_{rep}")
                nc.sync.dma_start(xt[:], x2t[j][:, :, u * 256:(u + 1) * 256])
                if not do_proj:
                    return
                k_ps = ps_pj.tile([128, 256], F32, tag="pj",
                                  name=f"kps{b}_{jj}_{u}_{rep}")
                for ec in range(EC):
                    nc.tensor.matmul(k_ps[:], wk_sb[:, ec], xt[:, ec],
                                     start=(ec == 0), stop=(ec == EC - 1))
                c0 = jj * 512 + u * 256
                nc.vector.tensor_copy(kt_sb[b][:, c0:c0 + 256], k_ps[:])
                # V in natural [kv, d] layout: lhsT = x2.T chunk (tokens move)
                for t in range(2):
                    kc = jj * 4 + u * 2 + t
                    v_ps = ps_pj.tile([128, 128], F32, tag="pj",
                                      name=f"vps{b}_{kc}_{rep}")
                    for ec in range(EC):
                        nc.tensor.matmul(
                            v_ps[:], xt[:, ec, t * 128:(t + 1) * 128],
                            wv_sb[:, ec],
                            start=(ec == 0), stop=(ec == EC - 1))
                    dst = v_sb[b][:, kc].rearrange("p (h x) -> p h x", h=2)
                    nc.vector.tensor_copy(
                        dst[:, :, 0:64],
                        v_ps[:].rearrange("p (h x) -> p h x", h=2))

            def scores_tile(b, g, h, kc, pt_full):
                gs0 = b * SQ + g * 512
                hp = h * 64
                s_ps = ps_s.tile([128, 1024], F32, tag="s",
                                 name=f"sps{b}_{g}_{kc}_{h}_{rep}")
                pt = work.tile([128, 1024], F16, tag="pt", bufs=34,
                               name=f"pt{b}_{g}_{kc}_{h}_{rep}")
                for u in range(2):
                    nc.tensor.matmul(
                        s_ps[:, u * 512:(u + 1) * 512],
                        kt_sb[b][hp:hp + 64,
                                 (kc + u) * 128:(kc + u + 1) * 128],
                        qt_sb[hp:hp + 64, gs0:gs0 + 512],
                        start=True, stop=True)
                nc.scalar.activation(pt[:], s_ps[:], Exp, scale=0.125)
                pt_full[(h, kc)] = pt

            def make_consumers(b, g, pt_full):
                # closures emitting the post-exp work of phase (b, g):
                # 8 o-sweeps (contiguous accum groups), 1 transpose step,
                # 8 out-projection chunks. Executed during phase (b,g)+1.
                anat = [work.tile([128, 128], F16, tag="anat", bufs=8,
                                  name=f"an{b}_{g}_{qc}_{rep}")
                        for qc in range(4)]

                def osweep(h, qc):
                    o2 = ps_o.tile([128, 65], F32, tag="o",
                                   name=f"o{b}_{g}_{h}_{qc}_{rep}")
                    for kvc in range(KVC_B):
                        pt = pt_full[(h, kvc & ~1)]
                        c0 = (kvc & 1) * 512 + qc * 128
                        nc.tensor.matmul(
                            o2[:], pt[:, c0:c0 + 128],
                            v_sb[b][:, kvc, h * 65:h * 65 + 65],
                            start=(kvc == 0), stop=(kvc == KVC_B - 1))
                    rec = work.tile([128, 1], F32, tag="rec", bufs=3,
                                    name=f"rc{b}_{g}_{h}_{qc}_{rep}")
                    nc.vector.reciprocal(rec[:], o2[:, 64:65])
                    nc.vector.tensor_scalar_mul(
                        anat[qc][:, h * 64:(h + 1) * 64],
                        o2[:, 0:64], rec[:])

                def at_step():
                    for qc in range(4):
                        atp = ps_pj.tile([128, 128], F16, tag="pj",
                                         name=f"atp{b}_{g}_{qc}_{rep}")
                        nc.tensor.transpose(atp[:], anat[qc][:], id_sb[:])
                        nc.vector.tensor_copy(
                            at_sb[b][:, g * 512 + qc * 128:
                                     g * 512 + (qc + 1) * 128],
                            atp[:])

                def oproj_o(o):
                    y_ps = ps_pj.tile([128, 512], F32, tag="pj",
                                      name=f"yps{b}_{g}_{o}_{rep}")
                    nc.tensor.matmul(y_ps[:], wo_sb[:, o * 128:(o + 1) * 128],
                                     at_sb[b][:, g * 512:(g + 1) * 512],
                                     start=True, stop=True)
                    y_sb = work.tile([128, 512], F16, tag="y", bufs=3,
                                     name=f"ysb{b}_{g}_{o}_{rep}")
                    nc.vector.tensor_copy(y_sb[:], y_ps[:])
                    nc.sync.dma_start(
                        yt_r[:, o, b * SQ + g * 512: b * SQ + (g + 1) * 512],
                        y_sb[:])

                cons = [(lambda h=h, qc=qc: osweep(h, qc))
                        for h in range(2) for qc in range(4)]
                cons.append(at_step)
                if do_oproj:
                    cons += [(lambda o=o: oproj_o(o)) for o in range(EC)]
                return cons

            def attn_phase(b, g, consumers, fill):
                # interleave: previous phase's consumers + projection quanta
                # ride inside this phase's scores/exp stream
                if not do_attn:
                    for f in list(consumers) + list(fill):
                        f()
                    return []
                consumers = list(consumers)
                fill = list(fill)
                pt_full = {}
                ci = fi = 0
                for ip in range(16):
                    h, kp = divmod(ip, 8)
                    while ci < len(consumers) and \
                            ci * 16 < (ip + 1) * len(consumers):
                        consumers[ci]()
                        ci += 1
                    while fi < len(fill) and fi * 16 < (ip + 1) * len(fill):
                        fill[fi]()
                        fi += 1
                    scores_tile(b, g, h, kp * 2, pt_full)
                for c in consumers[ci:]:
                    c()
                for f in fill[fi:]:
                    f()
                return make_consumers(b, g, pt_full)

            # lead-in: projections for batch 0 (+ first two q chunks),
            # Q and KV quanta interleaved so PE tracks the DMA stream
            lead = [(lambda u=u: proj_q(0, u)) for u in range(2)]
            lead += [(lambda u=u: proj_q(1, u)) for u in range(2)]
            kvq = [(lambda jj=jj, u=u: proj_kv(0, jj, u))
                   for jj in range(4) for u in range(2)]
            order = [lead[0], kvq[0], lead[1], kvq[1], lead[2], kvq[2],
                     lead[3], kvq[3]] + kvq[4:]
            emitted_wkv = False
            for i, qm in enumerate(order):
                qm()
                if not emitted_wkv:
                    nc.sync.dma_start(wk_sb[:], wkt[:])
                    nc.sync.dma_start(wv_sb[:], wvt[:])
                    emitted_wkv = True
                if i == 2:
                    nc.sync.dma_start(wo_sb[:], wot[:])
                    nc.sync.dma_start(id_sb[:], idv[:])

            cons = []
            for b in range(B):
                for g in range(2):
                    if b + 1 < B:
                        fl = [(lambda j=2 * b + 2 + g, u=u: proj_q(j, u))
                              for u in range(2)]
                        fl += [(lambda jj=jj, u=u: proj_kv(b + 1, jj, u))
                               for jj in (range(2) if g == 0 else range(2, 4))
                               for u in range(2)]
                    else:
                        fl = []
                    cons = attn_phase(b, g, cons, fl)
            for c in cons:
                c()

    nc.compile()
    return nc


def _get_nc(phases=("proj", "attn", "oproj"), n_reps=1):
    key = (tuple(phases), n_reps)
    if key not in _CACHE:
        _CACHE[key] = _build(phases, n_reps)
    return _CACHE[key]


def _tile_x(xt2d, nchunks):
    # [E, R] -> [R/512, 128, EC, 512]: x[j, p, ec, q] = xt2d[ec*128+p, j*512+q]
    return np.ascontiguousarray(
        xt2d.reshape(EC, 128, nchunks, 512).transpose(2, 1, 0, 3))


def _tile_w(wt_slice):
    # [E, 128] -> [128, EC, 128]
    return np.ascontiguousarray(
        wt_slice.reshape(EC, 128, 128).transpose(1, 0, 2))


def make_in_maps(x1, x2, Wq, bq, Wk, bk, Wv, bv, Wo, bo=None):
    f16 = np.float16
    x1 = np.asarray(x1, dtype=np.float32)
    x2 = np.asarray(x2, dtype=np.float32)
    x1t = _tile_x(np.ascontiguousarray(x1.reshape(Q_ROWS, E).T), QC).astype(f16)
    x2t = _tile_x(np.ascontiguousarray(x2.reshape(KV_ROWS, E).T),
                  KV_ROWS // 512).astype(f16)
    WqT = np.asarray(Wq, dtype=np.float32).T
    WkT = np.asarray(Wk, dtype=np.float32).T
    WvT = np.asarray(Wv, dtype=np.float32).T
    WoT = np.ascontiguousarray(np.asarray(Wo, dtype=np.float32).T)
    ident = np.eye(128, dtype=f16)
    in_maps = []
    for c in range(N_CORES):
        s = slice(128 * c, 128 * (c + 1))
        in_maps.append({
            "x1t": x1t, "x2t": x2t,
            "wqt": _tile_w(WqT[:, s]).astype(f16),
            "wkt": _tile_w(WkT[:, s]).astype(f16),
            "wvt": _tile_w(WvT[:, s]).astype(f16),
            "wot": np.ascontiguousarray(WoT[s, :]).astype(f16),
            "bq": np.ascontiguousarray(
                np.asarray(bq, np.float32)[s]).reshape(128, 1),
            "ident": ident,
        })
    return in_maps


def kernel(x1, x2, Wq, bq, Wk, bk, Wv, bv, Wo, bo):
    nc = _get_nc()
    in_maps = make_in_maps(x1, x2, Wq, bq, Wk, bk, Wv, bv, Wo)
    res = run_bass_kernel_spmd(nc, in_maps, list(range(N_CORES)))
    ytf = res.results[0]["yt"].astype(np.float32)
    for c in range(1, N_CORES):
        ytf += res.results[c]["yt"].astype(np.float32)
    # bv folds into the output bias: softmax rows sum to 1
    bo_eff = (np.asarray(bo, np.float64)
              + np.asarray(Wo, np.float64) @ np.asarray(bv, np.float64))
    y = ytf.T.astype(np.float32) + bo_eff.astype(np.float32)[None, :]
    return y.reshape(B, SQ, E)
